# revision 1
# baseline (speedup 1.0000x reference)
"""Grouped MoE (top-2 of 8 experts, SwiGLU) on 8 Trainium2 NeuronCores.

Sharding: expert-parallel. Core c owns expert c. Every core receives the
full activation tensor (transposed on host into feature-major layout),
computes the fp32 gate for all tokens on-device, runs its expert's three
GEMMs in bf16 over all tokens, scales rows by its own gate column, and the
partial outputs are combined with an on-device ReduceScatter; core c emits
output rows [c*512, (c+1)*512).

Host side does layout only: transposes, dtype casts, gate-weight column
permutation (so each core's own expert is column 0 uniformly), and final
concatenation of the output shards.
"""

import sys
import numpy as np

for _p in ("/opt/trn_rl_repo",):
    if _p not in sys.path:
        sys.path.insert(0, _p)

B, S, D, F, E = 2, 2048, 1024, 1024, 8
T = B * S            # 4096 tokens
NCORES = 8
TSH = T // NCORES    # 512 output rows per core
P = 128
TCH = 512            # token chunk in main loop
NTCH = T // TCH
NT = T // P          # 32 token tiles for the gate
DK = D // P          # 8 contraction chunks over D
FK = F // P          # 8 F tiles

_cache = {}


def _build_nc():
    from contextlib import ExitStack

    import concourse.bass as bass
    import concourse.mybir as mybir
    import concourse.tile as tile
    from concourse import bacc

    dt = mybir.dt
    AF = mybir.ActivationFunctionType
    ALU = mybir.AluOpType

    nc = bacc.Bacc("TRN2", target_bir_lowering=False, debug=False,
                   num_devices=NCORES)

    xt = nc.dram_tensor("xt", [D, T], dt.float32, kind="ExternalInput").ap()
    xb = nc.dram_tensor("xb", [D, T], dt.bfloat16, kind="ExternalInput").ap()
    w1t = nc.dram_tensor("w1t", [D, F], dt.bfloat16, kind="ExternalInput").ap()
    w3t = nc.dram_tensor("w3t", [D, F], dt.bfloat16, kind="ExternalInput").ap()
    w2t = nc.dram_tensor("w2t", [F, D], dt.bfloat16, kind="ExternalInput").ap()
    gwt = nc.dram_tensor("gwt", [D, E], dt.float32, kind="ExternalInput").ap()
    out = nc.dram_tensor("out", [TSH, D], dt.float32, kind="ExternalOutput").ap()

    with tile.TileContext(nc) as tc, ExitStack() as ctx:
        dram = ctx.enter_context(tc.tile_pool(name="dram", bufs=1, space="DRAM"))
        rsin = dram.tile([T, D], dt.bfloat16)
        rsout = dram.tile([TSH, D], dt.bfloat16)

        const = ctx.enter_context(tc.tile_pool(name="const", bufs=1))
        xpool = ctx.enter_context(tc.tile_pool(name="xpool", bufs=1))
        gstream = ctx.enter_context(tc.tile_pool(name="gstream", bufs=4))
        gsb = ctx.enter_context(tc.tile_pool(name="gsb", bufs=1))
        hpool = ctx.enter_context(tc.tile_pool(name="hpool", bufs=2))
        apool = ctx.enter_context(tc.tile_pool(name="apool", bufs=3))
        ypool = ctx.enter_context(tc.tile_pool(name="ypool", bufs=3))
        opool = ctx.enter_context(tc.tile_pool(name="opool", bufs=2))

        gpsum = ctx.enter_context(tc.tile_pool(name="gpsum", bufs=2, space="PSUM"))
        abpsum = ctx.enter_context(tc.tile_pool(name="abpsum", bufs=2, space="PSUM"))
        ypsum = ctx.enter_context(tc.tile_pool(name="ypsum", bufs=2, space="PSUM"))

        # ---- resident weights and activations ----
        w1_sb = []
        w3_sb = []
        w2_sb = []
        xb_sb = []
        for k in range(DK):
            t1 = const.tile([P, F], dt.bfloat16, tag=f"w1_{k}")
            nc.sync.dma_start(t1[:], w1t[k * P:(k + 1) * P, :])
            w1_sb.append(t1)
            t3 = const.tile([P, F], dt.bfloat16, tag=f"w3_{k}")
            nc.sync.dma_start(t3[:], w3t[k * P:(k + 1) * P, :])
            w3_sb.append(t3)
            t2 = const.tile([P, D], dt.bfloat16, tag=f"w2_{k}")
            nc.sync.dma_start(t2[:], w2t[k * P:(k + 1) * P, :])
            w2_sb.append(t2)
            tx = xpool.tile([P, T], dt.bfloat16, tag=f"xb_{k}")
            nc.sync.dma_start(tx[:], xb[k * P:(k + 1) * P, :])
            xb_sb.append(tx)

        gw_sb = []
        for k in range(DK):
            tg = const.tile([P, E], dt.float32, tag=f"gw_{k}")
            nc.sync.dma_start(tg[:], gwt[k * P:(k + 1) * P, :])
            gw_sb.append(tg)

        # ---- gate: fp32 logits -> top2 -> renormalized weight of own column ----
        max8 = gsb.tile([P, NT * 8], dt.float32, tag="max8")
        lme = gsb.tile([P, NT], dt.float32, tag="lme")
        for jb in range(NT // 4):
            xtg = []
            for k in range(DK):
                tgt = gstream.tile([P, 4 * P], dt.float32, tag=f"xtg{k % 2}")
                nc.sync.dma_start(
                    tgt[:], xt[k * P:(k + 1) * P, jb * 4 * P:(jb + 1) * 4 * P])
                xtg.append(tgt)
            for js in range(4):
                j = jb * 4 + js
                ps = gpsum.tile([P, 8], dt.float32, tag="gps")
                for k in range(DK):
                    nc.tensor.matmul(
                        ps[:],
                        lhsT=xtg[k][:, js * P:(js + 1) * P],
                        rhs=gw_sb[k][:],
                        start=(k == 0), stop=(k == DK - 1),
                    )
                nc.vector.max(out=max8[:, j * 8:(j + 1) * 8], in_=ps[:])
                nc.vector.tensor_copy(lme[:, j:j + 1], ps[:, 0:1])

        m8 = max8.rearrange("p (j e) -> p j e", e=8)
        m1 = m8[:, :, 0]
        m2 = m8[:, :, 1]
        tA = gsb.tile([P, NT], dt.float32, tag="tA")
        tB = gsb.tile([P, NT], dt.float32, tag="tB")
        sel = gsb.tile([P, NT], dt.float32, tag="sel")
        gpk = gsb.tile([P, NT], dt.float32, tag="gpk")
        # sel = (l_own >= m2)
        nc.vector.tensor_tensor(sel[:], lme[:], m2, op=ALU.is_ge)
        # tA = exp(l_own - m1)
        nc.vector.tensor_tensor(tA[:], lme[:], m1, op=ALU.subtract)
        nc.scalar.activation(tA[:], tA[:], AF.Exp)
        # tB = 1 + exp(m2 - m1)
        nc.vector.tensor_tensor(tB[:], m2, m1, op=ALU.subtract)
        nc.scalar.activation(tB[:], tB[:], AF.Exp)
        nc.vector.tensor_scalar_add(tB[:], tB[:], 1.0)
        nc.vector.reciprocal(tB[:], tB[:])
        # g = sel * exp(l-m1) / (1 + exp(m2-m1))
        nc.vector.tensor_tensor(gpk[:], tA[:], tB[:], op=ALU.mult)
        nc.vector.tensor_tensor(gpk[:], gpk[:], sel[:], op=ALU.mult)

        # ---- dense expert compute over token chunks ----
        for tci in range(NTCH):
            tok = tci * TCH
            h_sb = []
            for f in range(FK):
                psA = abpsum.tile([P, TCH], dt.float32, tag="psA")
                psB = abpsum.tile([P, TCH], dt.float32, tag="psB")
                for k in range(DK):
                    nc.tensor.matmul(
                        psA[:], lhsT=w1_sb[k][:, f * P:(f + 1) * P],
                        rhs=xb_sb[k][:, tok:tok + TCH],
                        start=(k == 0), stop=(k == DK - 1))
                for k in range(DK):
                    nc.tensor.matmul(
                        psB[:], lhsT=w3_sb[k][:, f * P:(f + 1) * P],
                        rhs=xb_sb[k][:, tok:tok + TCH],
                        start=(k == 0), stop=(k == DK - 1))
                asb = apool.tile([P, TCH], dt.float32, tag="asb")
                nc.scalar.activation(asb[:], psA[:], AF.Sigmoid)
                tsb = apool.tile([P, TCH], dt.float32, tag="tsb")
                nc.vector.tensor_tensor(tsb[:], asb[:], psA[:], op=ALU.mult)
                hsb = hpool.tile([P, TCH], dt.bfloat16, tag=f"h{f}")
                nc.vector.tensor_tensor(hsb[:], tsb[:], psB[:], op=ALU.mult)
                h_sb.append(hsb)
            for m in range(TCH // P):
                jj = tci * (TCH // P) + m
                for nhalf in range(2):
                    psY = ypsum.tile([P, 512], dt.float32, tag="psY")
                    for fk in range(FK):
                        nc.tensor.matmul(
                            psY[:],
                            lhsT=h_sb[fk][:, m * P:(m + 1) * P],
                            rhs=w2_sb[fk][:, nhalf * 512:(nhalf + 1) * 512],
                            start=(fk == 0), stop=(fk == FK - 1))
                    ysb = ypool.tile([P, 512], dt.bfloat16, tag="ysb")
                    nc.scalar.activation(ysb[:], psY[:], AF.Copy,
                                         scale=gpk[:, jj:jj + 1])
                    nc.gpsimd.dma_start(
                        rsin[tok + m * P: tok + (m + 1) * P,
                             nhalf * 512:(nhalf + 1) * 512],
                        ysb[:])

        # ---- combine across cores ----
        nc.gpsimd.collective_compute(
            "ReduceScatter",
            ALU.add,
            ins=[rsin.opt()],
            outs=[rsout.opt()],
            replica_groups=[list(range(NCORES))],
        )
        for m in range(TSH // P):
            ob = opool.tile([P, D], dt.bfloat16, tag="ob")
            nc.sync.dma_start(ob[:], rsout[m * P:(m + 1) * P, :])
            of = opool.tile([P, D], dt.float32, tag="of")
            nc.vector.tensor_copy(of[:], ob[:])
            nc.sync.dma_start(out[m * P:(m + 1) * P, :], of[:])

    nc.compile()
    return nc


def xtile_gate(nc, pool, xt, dt, k, j):
    t = pool.tile([P, P], dt.float32, tag="xtg")
    nc.sync.dma_start(t[:], xt[k * P:(k + 1) * P, j * P:(j + 1) * P])
    return t[:]


def kernel(x, gate_w, w1, w3, w2):
    import ml_dtypes
    from concourse.bass_utils import run_bass_kernel_spmd

    xf = np.ascontiguousarray(x.reshape(T, D).astype(np.float32))
    xT = np.ascontiguousarray(xf.T)                       # [D, T] f32
    xTb = xT.astype(ml_dtypes.bfloat16)                   # [D, T] bf16

    if "nc" not in _cache:
        _cache["nc"] = _build_nc()
    nc = _cache["nc"]

    in_maps = []
    for c in range(NCORES):
        perm = [c] + [e for e in range(E) if e != c]
        gwt_c = np.ascontiguousarray(gate_w[perm].T.astype(np.float32))  # [D, E]
        in_maps.append({
            "xt": xT,
            "xb": xTb,
            "w1t": np.ascontiguousarray(w1[c].T).astype(ml_dtypes.bfloat16),
            "w3t": np.ascontiguousarray(w3[c].T).astype(ml_dtypes.bfloat16),
            "w2t": np.ascontiguousarray(w2[c].T).astype(ml_dtypes.bfloat16),
            "gwt": gwt_c,
        })

    res = run_bass_kernel_spmd(nc, in_maps, list(range(NCORES)))
    shards = [res.results[c]["out"] for c in range(NCORES)]
    outf = np.concatenate(shards, axis=0).astype(np.float32)
    return outf.reshape(B, S, D)



# revision 2
# speedup vs baseline: 5.2804x; 5.2804x over previous
"""Grouped MoE (top-2 of 8 experts, SwiGLU) on 8 Trainium2 NeuronCores.

Sharding: expert-parallel with real token dispatch. The top-2 gate is
computed on host (33 MFLOP of numpy, exactly reproducing the reference's
softmax/top-k math); tokens are gathered per expert on host. Core c owns
expert c and runs the three expert GEMMs in bf16 over only the tokens
routed to expert c (padded to a fixed capacity C, a multiple of 128),
scales each output row by the host-computed renormalized gate weight,
and writes its [C, D] partial output. The host scatter-adds the two
expert partials per token into the full [T, D] fp32 output.

No collectives: each (token, expert) pair is computed on exactly one
core, so combining is a disjoint scatter-add on host.
"""

import sys
import numpy as np

for _p in ("/opt/trn_rl_repo",):
    if _p not in sys.path:
        sys.path.insert(0, _p)

B, S, D, F, E = 2, 2048, 1024, 1024, 8
T = B * S            # 4096 tokens
NCORES = 8
P = 128
DK = D // P          # 8 contraction chunks over D
FK = F // P          # 8 F tiles

_cache = {}


def _build_nc(C):
    """Expert kernel over C routed tokens (C a multiple of 128)."""
    from contextlib import ExitStack

    import concourse.mybir as mybir
    import concourse.tile as tile
    from concourse import bacc

    dt = mybir.dt
    AF = mybir.ActivationFunctionType
    ALU = mybir.AluOpType

    NTT = C // P         # token tiles
    # token chunks of up to 512 (PSUM free-dim limit)
    chunks = []
    t = 0
    while t < C:
        c = min(512, C - t)
        chunks.append((t, c))
        t += c

    nc = bacc.Bacc("TRN2", target_bir_lowering=False, debug=False,
                   num_devices=NCORES)

    xg = nc.dram_tensor("xg", [D, C], dt.bfloat16, kind="ExternalInput").ap()
    w1t = nc.dram_tensor("w1t", [D, F], dt.bfloat16, kind="ExternalInput").ap()
    w3t = nc.dram_tensor("w3t", [D, F], dt.bfloat16, kind="ExternalInput").ap()
    w2t = nc.dram_tensor("w2t", [F, D], dt.bfloat16, kind="ExternalInput").ap()
    gs = nc.dram_tensor("gs", [P, NTT], dt.float32, kind="ExternalInput").ap()
    out = nc.dram_tensor("out", [C, D], dt.bfloat16, kind="ExternalOutput").ap()

    with tile.TileContext(nc) as tc, ExitStack() as ctx:
        const = ctx.enter_context(tc.tile_pool(name="const", bufs=1))
        hpool = ctx.enter_context(tc.tile_pool(name="hpool", bufs=2))
        apool = ctx.enter_context(tc.tile_pool(name="apool", bufs=3))
        ypool = ctx.enter_context(tc.tile_pool(name="ypool", bufs=3))

        abpsum = ctx.enter_context(tc.tile_pool(name="abpsum", bufs=2, space="PSUM"))
        ypsum = ctx.enter_context(tc.tile_pool(name="ypsum", bufs=2, space="PSUM"))

        # ---- resident weights and routed activations ----
        gs_sb = const.tile([P, NTT], dt.float32, tag="gs")
        nc.sync.dma_start(gs_sb[:], gs[:, :])
        w1_sb = []
        w3_sb = []
        w2_sb = []
        xg_sb = []
        for k in range(DK):
            t1 = const.tile([P, F], dt.bfloat16, tag=f"w1_{k}")
            nc.sync.dma_start(t1[:], w1t[k * P:(k + 1) * P, :])
            w1_sb.append(t1)
            t3 = const.tile([P, F], dt.bfloat16, tag=f"w3_{k}")
            nc.sync.dma_start(t3[:], w3t[k * P:(k + 1) * P, :])
            w3_sb.append(t3)
            tx = const.tile([P, C], dt.bfloat16, tag=f"xg_{k}")
            nc.sync.dma_start(tx[:], xg[k * P:(k + 1) * P, :])
            xg_sb.append(tx)
        for k in range(FK):
            t2 = const.tile([P, D], dt.bfloat16, tag=f"w2_{k}")
            nc.sync.dma_start(t2[:], w2t[k * P:(k + 1) * P, :])
            w2_sb.append(t2)

        # ---- expert compute over token chunks ----
        for (tok, tch) in chunks:
            h_sb = []
            for f in range(FK):
                psA = abpsum.tile([P, tch], dt.float32, tag="psA")
                psB = abpsum.tile([P, tch], dt.float32, tag="psB")
                for k in range(DK):
                    nc.tensor.matmul(
                        psA[:], lhsT=w1_sb[k][:, f * P:(f + 1) * P],
                        rhs=xg_sb[k][:, tok:tok + tch],
                        start=(k == 0), stop=(k == DK - 1))
                for k in range(DK):
                    nc.tensor.matmul(
                        psB[:], lhsT=w3_sb[k][:, f * P:(f + 1) * P],
                        rhs=xg_sb[k][:, tok:tok + tch],
                        start=(k == 0), stop=(k == DK - 1))
                asb = apool.tile([P, tch], dt.float32, tag="asb")
                nc.scalar.activation(asb[:], psA[:], AF.Sigmoid)
                tsb = apool.tile([P, tch], dt.float32, tag="tsb")
                nc.vector.tensor_tensor(tsb[:], asb[:], psA[:], op=ALU.mult)
                hsb = hpool.tile([P, tch], dt.bfloat16, tag=f"h{f}")
                nc.vector.tensor_tensor(hsb[:], tsb[:], psB[:], op=ALU.mult)
                h_sb.append(hsb)
            for m in range(tch // P):
                jj = tok // P + m
                ysb = ypool.tile([P, D], dt.bfloat16, tag="ysb")
                for nhalf in range(2):
                    psY = ypsum.tile([P, 512], dt.float32, tag="psY")
                    for fk in range(FK):
                        nc.tensor.matmul(
                            psY[:],
                            lhsT=h_sb[fk][:, m * P:(m + 1) * P],
                            rhs=w2_sb[fk][:, nhalf * 512:(nhalf + 1) * 512],
                            start=(fk == 0), stop=(fk == FK - 1))
                    nc.scalar.activation(ysb[:, nhalf * 512:(nhalf + 1) * 512],
                                         psY[:], AF.Copy,
                                         scale=gs_sb[:, jj:jj + 1])
                nc.gpsimd.dma_start(
                    out[tok + m * P: tok + (m + 1) * P, :], ysb[:])

    nc.compile()
    return nc


def _route(x, gate_w):
    """Host gate: top-2 of 8, renormalized weights; per-expert token lists."""
    xf = np.ascontiguousarray(x.reshape(T, D).astype(np.float32))
    logits = xf @ gate_w.T.astype(np.float32)            # [T, E]
    order = np.argsort(-logits, axis=1, kind="stable")
    i1 = order[:, 0]
    i2 = order[:, 1]
    ar = np.arange(T)
    l1 = logits[ar, i1]
    l2 = logits[ar, i2]
    g1 = 1.0 / (1.0 + np.exp(l2 - l1))                   # renormalized top-2
    g2 = 1.0 - g1
    idxs, gws = [], []
    for e in range(E):
        m1 = i1 == e
        m2 = i2 == e
        idx = np.nonzero(m1 | m2)[0]
        g = np.where(m1, g1, g2)[idx].astype(np.float32)
        idxs.append(idx)
        gws.append(g)
    maxn = max(len(i) for i in idxs)
    C = max(512, -(-maxn // P) * P)
    return xf, idxs, gws, C


def _prepare(x, gate_w, w1, w3, w2):
    """Build (nc, in_maps, route_meta) for an SPMD run."""
    import ml_dtypes

    xf, idxs, gws, C = _route(x, gate_w)
    xTb = np.ascontiguousarray(xf.T).astype(ml_dtypes.bfloat16)  # [D, T]

    key = ("nc", C)
    if key not in _cache:
        _cache[key] = _build_nc(C)
    nc = _cache[key]

    in_maps = []
    for c in range(NCORES):
        idx = idxs[c]
        n = len(idx)
        xg = np.zeros((D, C), dtype=ml_dtypes.bfloat16)
        xg[:, :n] = xTb[:, idx]
        gpad = np.zeros(C, dtype=np.float32)
        gpad[:n] = gws[c]
        gs = np.ascontiguousarray(gpad.reshape(C // P, P).T)     # [P, NTT]
        in_maps.append({
            "xg": xg,
            "w1t": np.ascontiguousarray(w1[c].T).astype(ml_dtypes.bfloat16),
            "w3t": np.ascontiguousarray(w3[c].T).astype(ml_dtypes.bfloat16),
            "w2t": np.ascontiguousarray(w2[c].T).astype(ml_dtypes.bfloat16),
            "gs": gs,
        })
    return nc, in_maps, (idxs, C)


def _combine(results, meta):
    idxs, C = meta
    outf = np.zeros((T, D), dtype=np.float32)
    for e in range(E):
        idx = idxs[e]
        y = np.asarray(results[e]["out"])[:len(idx)].astype(np.float32)
        outf[idx] += y
    return outf.reshape(B, S, D)


def kernel(x, gate_w, w1, w3, w2):
    from concourse.bass_utils import run_bass_kernel_spmd

    nc, in_maps, meta = _prepare(x, gate_w, w1, w3, w2)
    res = run_bass_kernel_spmd(nc, in_maps, list(range(NCORES)))
    return _combine(res.results, meta)
